# revision 29
# baseline (speedup 1.0000x reference)
"""LinOSS layer Trainium2 kernel.

Math: the per-state 2x2 recurrence matrix M = [[1, -sA], [s, 1-s^2 A]] has
det(M)=1 and eigenvalues e^{+-i theta} with cos(theta) = 1 - s^2 A / 2, so
M^d = p_d M - p_{d-1} I with p_d = sin(d theta)/sin(theta).  The scanned state
x_t collapses to a rank-2 modulated prefix sum:

    u_t   = s * Bu_t            (s folded into B on host)
    T1    = gamma*cos(t th) + sin(t th);  T2 = cos(t th) - gamma*sin(t th)
    E     = cumsum(T1 * u);     F = cumsum(T2 * u)
    x_t   = sin(t th) * E_t + cos(t th) * F_t
    gamma = (s - s^2 A / 2) / sin(theta)

Sharding: states P=256 split across 8 cores (32 each); inside a core, time
L=8192 is folded 4x into partitions -> tiles are (128=[4 chunks x 32 states],
2048).  Fold-chunk carries are fixed with per-partition cumsum offsets
(strictly-lower chunk mask matmul).  Each core emits a partial (H, L) output
(its 32-state slice of ys @ C^T, plus input*D on core 0 only); the host sums
partials and transposes - that is the unshard/all-reduce step for this
sharding.
"""

import numpy as np

L, H, P = 8192, 128, 256
NCORES = 8
SLOC = P // NCORES          # states per core
FOLD = 4                    # time chunks folded into partitions
CL = L // FOLD              # 2048 free columns per partition row
NPART = FOLD * SLOC         # 128
SEED = 128                  # host-seeded table width
DOUBLINGS = [128, 256, 512, 1024]
JT = 512                    # j-tile width (psum bank)
NJT = CL // JT              # 4
NTT = L // 128              # 64 transpose tiles

_CACHE: dict = {}


def _build_bass(split_waits=True):
    import concourse.bass as bass
    import concourse.mybir as mybir
    import concourse.tile as tile
    from concourse.masks import make_identity

    dt = mybir.dt.float32
    bt = mybir.dt.bfloat16
    Alu = mybir.AluOpType

    nc = bass.Bass(
        trn_type="TRN2",
        target_bir_lowering=False,
        debug=False,
        num_devices=NCORES,
    )

    inp = nc.dram_tensor("inp", [L, H], bt, kind="ExternalInput").ap()
    Bt_d = nc.dram_tensor("Bt", [H, 2 * SLOC], bt, kind="ExternalInput").ap()
    Ctr_d = nc.dram_tensor("Ctr", [NPART, H], bt, kind="ExternalInput").ap()
    Cti_d = nc.dram_tensor("Cti", [NPART, H], bt, kind="ExternalInput").ap()
    dD_d = nc.dram_tensor("dD", [H, H], bt, kind="ExternalInput").ap()
    Wm_d = nc.dram_tensor("Wm", [NPART, NPART], dt, kind="ExternalInput").ap()
    consts_d = nc.dram_tensor("consts", [NPART, 16], dt, kind="ExternalInput").ap()
    seedS_d = nc.dram_tensor("seedS", [NPART, SEED], bt, kind="ExternalInput").ap()
    seedC_d = nc.dram_tensor("seedC", [NPART, SEED], bt, kind="ExternalInput").ap()
    outp = nc.dram_tensor("outp", [H, L], dt, kind="ExternalOutput").ap()

    with tile.TileContext(nc) as tc:
        cpool = tc.alloc_tile_pool(name="const", bufs=1)
        big1 = tc.alloc_tile_pool(name="big1", bufs=1)
        work = tc.alloc_tile_pool(name="work", bufs=2)
        evac = tc.alloc_tile_pool(name="evac", bufs=2)
        psum = tc.alloc_tile_pool(name="psum", bufs=2, space="PSUM")
        psum_bu = tc.alloc_tile_pool(name="psum_bu", bufs=2, space="PSUM")
        big2 = tc.alloc_tile_pool(name="big2", bufs=1)

        consts = cpool.tile_from(consts_d)
        inpT = big1.tile([128, L], bt, tag="inpT")
        for q in range(FOLD):
            nc.sync.dma_start_transpose(
                out=inpT[:, q * CL : (q + 1) * CL],
                in_=inp[q * CL : (q + 1) * CL, :],
            )
        Bt = cpool.tile_from(Bt_d)
        Ctr = cpool.tile_from(Ctr_d)
        Cti = cpool.tile_from(Cti_d)
        dD = cpool.tile_from(dD_d)
        Wm = cpool.tile_from(Wm_d)
        ones = cpool.tile([NPART, CL], dt)
        nc.vector.memset(ones[:], 1.0)


        gamma = consts[:, 0:1]
        gamma_neg = consts[:, 1:2]
        cosD = [consts[:, 2 + m : 3 + m] for m in range(4)]
        sinD = [consts[:, 6 + m : 7 + m] for m in range(4)]

        # ---- sin/cos tables (global angles), built by angle-doubling ----
        sinT = big1.tile([NPART, CL], bt, tag="sinT")
        cosT = big1.tile([NPART, CL], bt, tag="cosT")
        nc.sync.dma_start(out=sinT[:, 0:SEED], in_=seedS_d)
        nc.sync.dma_start(out=cosT[:, 0:SEED], in_=seedC_d)
        n = SEED
        for m, nn_ in enumerate(DOUBLINGS):
            assert nn_ == n
            t1 = work.tile([NPART, n], bt, tag="tbl")
            t2 = work.tile([NPART, n], bt, tag="tbl")
            # sin(x+D) = sin x cos D + cos x sin D
            nc.scalar.activation(
                t1[:], cosT[:, 0:n], mybir.ActivationFunctionType.Copy,
                scale=sinD[m],
            )
            nc.vector.scalar_tensor_tensor(
                sinT[:, n : 2 * n], sinT[:, 0:n], cosD[m], t1[:],
                Alu.mult, Alu.add,
            )
            # cos(x+D) = cos x cos D - sin x sin D
            nc.scalar.activation(
                t2[:], sinT[:, 0:n], mybir.ActivationFunctionType.Copy,
                scale=sinD[m],
            )
            nc.vector.scalar_tensor_tensor(
                cosT[:, n : 2 * n], cosT[:, 0:n], cosD[m], t2[:],
                Alu.mult, Alu.subtract,
            )
            n *= 2
        assert n == CL

        # ---- input load (tiled) + on-chip transpose to (H x L) ----
        T1 = big2.tile([NPART, CL], bt, tag="T1")
        T2 = big2.tile([NPART, CL], bt, tag="T2")
        nc.vector.scalar_tensor_tensor(
            T1[:], cosT[:], gamma, sinT[:], Alu.mult, Alu.add
        )
        nc.vector.scalar_tensor_tensor(
            T2[:], sinT[:], gamma_neg, cosT[:], Alu.mult, Alu.add
        )

        # ---- Bu matmuls + modulation + chained scans ----
        Er = big2.tile([NPART, CL], dt, tag="Er")
        Fr = big2.tile([NPART, CL], dt, tag="Fr")
        Ei = big2.tile([NPART, CL], dt, tag="Ei")
        Fi = big2.tile([NPART, CL], dt, tag="Fi")
        EFs = [Er, Fr, Ei, Fi]
        Y1r = big2.tile([NPART, CL], dt, tag="Y1r")
        Y2r = big2.tile([NPART, CL], dt, tag="Y2r")
        Y1i = big2.tile([NPART, CL], dt, tag="Y1i")
        Y2i = big2.tile([NPART, CL], dt, tag="Y2i")


        for jt in range(NJT):
            js = slice(jt * JT, (jt + 1) * JT)
            pbu_r = psum_bu.tile([NPART, JT], dt, tag="bu_r")
            pbu_i = psum_bu.tile([NPART, JT], dt, tag="bu_i")
            for c in range(FOLD):
                rhs = inpT[:, c * CL + jt * JT : c * CL + (jt + 1) * JT]
                ps = slice(c * SLOC, (c + 1) * SLOC)
                nc.tensor.matmul(
                    pbu_r[ps, :], Bt[:, 0:SLOC], rhs, start=True, stop=True,
                    tile_position=(0, c * SLOC),
                )
                nc.tensor.matmul(
                    pbu_i[ps, :], Bt[:, SLOC : 2 * SLOC], rhs,
                    start=True, stop=True,
                    tile_position=(0, c * SLOC),
                )
            u_r = evac.tile([NPART, JT], bt, tag="u_r")
            u_i = evac.tile([NPART, JT], bt, tag="u_i")
            nc.scalar.copy(u_r[:], pbu_r[:])
            nc.scalar.copy(u_i[:], pbu_i[:])
            nc.vector.tensor_mul(Y1r[:, js], u_r[:], T1[:, js])
            nc.gpsimd.tensor_mul(Y2r[:, js], u_r[:], T2[:, js])
            nc.vector.tensor_mul(Y1i[:, js], u_i[:], T1[:, js])
            nc.gpsimd.tensor_mul(Y2i[:, js], u_i[:], T2[:, js])

        for arr, y in zip(EFs, [Y1r, Y2r, Y1i, Y2i]):
            # builder lives on BassGpSimd, but TRN2 runs the scan on DVE
            bass.BassGpSimd.tensor_tensor_scan(
                nc.vector, arr[:], ones[:], y[:], 0.0, Alu.mult, Alu.add
            )

        # ---- fold-chunk carry offsets ----
        fins = cpool.tile([NPART, 4], dt)
        for i, arr in enumerate(EFs):
            nc.scalar.copy(fins[:, i : i + 1], arr[:, CL - 1 : CL])
        poff = psum.tile([NPART, 4], dt, tag="out")
        nc.tensor.matmul(poff[:], Wm[:], fins[:], start=True, stop=True)
        offs = cpool.tile([NPART, 4], dt)
        nc.scalar.copy(offs[:], poff[:])

        # ---- demodulate + project + D-term + store ----
        for jt in range(NJT):
            js = slice(jt * JT, (jt + 1) * JT)
            eEr = work.tile([NPART, JT], bt, tag="w0")
            eFr = work.tile([NPART, JT], bt, tag="w1")
            eEi = work.tile([NPART, JT], bt, tag="w2")
            eFi = work.tile([NPART, JT], bt, tag="w3")
            Ident = mybir.ActivationFunctionType.Identity
            nc.scalar.activation(eEr[:], Er[:, js], Ident, bias=offs[:, 0:1])
            nc.scalar.activation(eFr[:], Fr[:, js], Ident, bias=offs[:, 1:2])
            nc.scalar.activation(eEi[:], Ei[:, js], Ident, bias=offs[:, 2:3])
            nc.scalar.activation(eFi[:], Fi[:, js], Ident, bias=offs[:, 3:4])
            t1r = work.tile([NPART, JT], bt, tag="w4")
            t2r = work.tile([NPART, JT], bt, tag="w5")
            t1i = work.tile([NPART, JT], bt, tag="w6")
            t2i = work.tile([NPART, JT], bt, tag="w7")
            x_r = work.tile([NPART, JT], bt, tag="w8")
            x_i = work.tile([NPART, JT], bt, tag="w9")
            nc.vector.tensor_mul(t1r[:], eEr[:], sinT[:, js])
            nc.gpsimd.tensor_mul(t2r[:], eFr[:], cosT[:, js])
            nc.vector.tensor_mul(t1i[:], eEi[:], sinT[:, js])
            nc.gpsimd.tensor_mul(t2i[:], eFi[:], cosT[:, js])
            nc.vector.tensor_add(x_r[:], t1r[:], t2r[:])
            nc.gpsimd.tensor_add(x_i[:], t1i[:], t2i[:])
            for c in range(FOLD):
                ps = slice(c * SLOC, (c + 1) * SLOC)
                po = psum.tile([128, JT], dt, tag="out")
                nc.tensor.matmul(
                    po[:], Ctr[ps, :], x_r[ps, :], start=True, stop=False,
                    tile_position=(c * SLOC, 0),
                )
                nc.tensor.matmul(
                    po[:], Cti[ps, :], x_i[ps, :],
                    start=False, stop=False,
                    tile_position=(c * SLOC, 0),
                )
                nc.tensor.matmul(
                    po[:], dD[:],
                    inpT[:, c * CL + jt * JT : c * CL + (jt + 1) * JT],
                    start=False, stop=True,
                )
                osb = evac.tile([128, JT], dt, tag="osb")
                nc.scalar.copy(osb[:], po[:])
                nc.sync.dma_start(
                    out=outp[:, c * CL + jt * JT : c * CL + (jt + 1) * JT],
                    in_=osb[:],
                )
        for p in (big2, psum_bu, psum, evac, work, big1, cpool):
            p.release()
    if split_waits:
        _split_matmul_waits(nc, mybir)
    return nc


def _split_matmul_waits(nc, mybir):
    """Hardware instruction structs fit a limited number of embedded sync
    waits (1 for the fp32 self-loading LDWEIGHTS matmul, 2 for ACT/DVE/POOL
    compute structs); move extra waits onto an inserted same-queue no-op."""
    caps = {"InstMatmult": 1}
    skip = {"InstNoOp", "InstAllEngineBarrier", "InstSync"}
    k = 0
    for bb in nc.main_func.blocks:
        insts = bb.instructions
        i = 0
        while i < len(insts):
            ins = insts[i]
            tn = type(ins).__name__
            if tn not in skip and ins.sync_info is not None:
                cap = caps.get(tn, 1)
                w = list(ins.sync_info.on_wait or [])
                if len(w) > cap:
                    for wj in w[:-cap]:
                        nop = mybir.InstNoOp(
                            name=f"I-mmdep-{k}",
                            engine=ins.engine,
                            ins=[],
                            outs=[],
                            sync_info=mybir.SyncInfo(
                                on_wait=[wj], on_update=[]
                            ),
                        )
                        k += 1
                        insts.insert(i, nop)
                        i += 1
                    ins.sync_info = mybir.SyncInfo(
                        on_wait=w[-cap:], on_update=ins.sync_info.on_update
                    )
            i += 1


def _host_prep(inputs):
    import ml_dtypes
    inp = np.ascontiguousarray(
        np.asarray(inputs["input_sequence"], np.float32).astype(ml_dtypes.bfloat16)
    )
    A = np.maximum(np.asarray(inputs["A_diag_raw"], np.float64), 0.0)
    s = 1.0 / (1.0 + np.exp(-np.asarray(inputs["steps_raw"], np.float64)))
    Br = np.asarray(inputs["B_real"], np.float64)
    Bi = np.asarray(inputs["B_img"], np.float64)
    Cr = np.asarray(inputs["C_real"], np.float64)
    Ci = np.asarray(inputs["C_img"], np.float64)
    D = np.asarray(inputs["D"], np.float64)

    costh = 1.0 - s * s * A / 2.0
    sinth = np.sqrt(np.maximum(1.0 - costh * costh, 1e-300))
    theta = np.arctan2(sinth, costh)
    gamma = (s - s * s * A / 2.0) / sinth

    import ml_dtypes
    f32 = np.float32
    bf16 = ml_dtypes.bfloat16
    in_maps = []
    twopi = 2.0 * np.pi
    for k in range(NCORES):
        sl = slice(k * SLOC, (k + 1) * SLOC)
        th = theta[sl]  # (SLOC,)
        Bt = np.empty((H, 2 * SLOC), bf16)
        Bt[:, 0:SLOC] = (s[sl, None] * Br[sl]).T.astype(bf16)
        Bt[:, SLOC:] = (s[sl, None] * Bi[sl]).T.astype(bf16)
        Ctr = np.tile(Cr[:, sl].T, (FOLD, 1)).astype(bf16)
        Cti = np.tile(-Ci[:, sl].T, (FOLD, 1)).astype(bf16)
        dD = (np.diag(D) if k == 0 else np.zeros((H, H))).astype(bf16)

        # per-partition q = c*SLOC + s
        th_q = np.tile(th, FOLD)  # (NPART,)
        tbase = np.repeat(np.arange(FOLD) * CL, SLOC).astype(np.float64)
        consts = np.zeros((NPART, 16), f32)
        consts[:, 0] = np.tile(gamma[sl], FOLD)
        consts[:, 1] = -consts[:, 0]
        for m, n in enumerate(DOUBLINGS):
            ang = np.mod(n * th_q, twopi)
            consts[:, 2 + m] = np.cos(ang)
            consts[:, 6 + m] = np.sin(ang)
        j = np.arange(SEED, dtype=np.float64)
        ang0 = np.mod((tbase[:, None] + j[None, :]) * th_q[:, None], twopi)
        seedS = np.sin(ang0).astype(bf16)
        seedC = np.cos(ang0).astype(bf16)

        q = np.arange(NPART)
        Wm = ((q[:, None] % SLOC == q[None, :] % SLOC)
              & (q[:, None] // SLOC < q[None, :] // SLOC)).astype(f32)

        in_maps.append({
            "inp": inp,
            "Bt": Bt,
            "Ctr": Ctr,
            "Cti": Cti,
            "dD": dD,
            "Wm": Wm,
            "consts": consts,
            "seedS": seedS,
            "seedC": seedC,
        })
    return in_maps


LAST_RESULTS = None


def kernel(**inputs) -> np.ndarray:
    global LAST_RESULTS
    from concourse.bass_utils import run_bass_kernel_spmd

    if "nc" not in _CACHE:
        _CACHE["nc"] = _build_bass()
    nc = _CACHE["nc"]

    in_maps = _host_prep(inputs)
    res = run_bass_kernel_spmd(nc, in_maps, core_ids=list(range(NCORES)))
    LAST_RESULTS = res
    part = np.zeros((H, L), np.float32)
    for r in res.results:
        part += r["outp"]
    return np.ascontiguousarray(part.T)
